# revision 60
# baseline (speedup 1.0000x reference)
"""Trainium2 Bass kernel for nn_HadamardBlock (GNN message passing block).

Reference computation (see reference.py):
    h_res = residual_layer(h, w_pre0, w_pre1)            # (nAtoms, E)
    mlp_bf = bf @ w_bf                                   # (nEdges, E)
    x = h_res[idx_s] * mlp_bf                            # gather + Hadamard
    x2 = segment_sum(x, idx_t, nAtoms) * scale_sum
    out = MLP(x2)   # Dense+ScaledSiLU then 3 residual blocks

Distribution strategy (8 cores, SPMD):
  - Edges are sharded by OWNER OF TARGET ATOM (atom ranges of 6250/core),
    so segment_sum is fully core-local and the atom MLP is data-parallel.
  - Phase 1 (h_res table) is sharded: each core computes 13 of the 104
    padded 512-atom tiles and an HBM-HBM AllGather replicates the full
    (53248, 128) bf16 table to every core.
  - Edge features ship as BITS-bit packed words (midrise quantizer
    q=floor(bf*2^B); device computes 2q+1 exactly and 1/2^(B+1) is folded
    into w_bf) and are unpacked with shift/mask DVE ops on device.
  - h ships at 10 bits (int8 high part + packed 2-bit low plane, scale
    Q10=160 on h^T/S, clipping beyond ~5.3 sigma).
  - The source gather h_res[idx_s] uses DMA gather (int16 indices; the
    table is addressed in two halves split at row 32768, and each core's
    edge stream is grouped low-half-first so indices fit in int16).
  - segment_sum runs on the tensor engine as x2^T += x^T @ onehot over
    128-atom windows; onehots for up to 4 (block, window) pairs at a time
    come from one DVE tensor_tensor(is_equal) with stride-0 broadcast APs.
  - Atoms are assigned to (core, window) bins by balanced LPT binning on
    in-degree with rank-matching on low-half degree, and each core's edge
    stream is packed CONTIGUOUSLY per (core, seg) -- windows back-to-back
    with no per-window padding (~0.2% total padding).  Window boundaries
    then differ per core, so the scatter uses a static conservative
    (block, window) pair schedule (union over cores) while the shipped
    per-core tcol masks (-1 = no match) select each core's real edges.
  - All per-core inputs are packed into a SINGLE "blob" tensor so the run
    costs one device_put (per-put overhead is ~80 ms here); the replicated
    weights/constants ship as 1/8 slices and are AllGather'd on device.
  - The output ships as per-feature-scaled 10-bit planes (int8 hi +
    packed 2-bit lo + absmax f32), 0.625x the bytes of bf16.
  - Capacities and the pair schedule are data-driven; the program is
    rebuilt per call, which the fast walrus BIR->NEFF compiler makes
    cheap (~0.3 s).

Everything is sized to minimize bytes shipped through the axon tunnel:
host->device upload is the dominant cost of a run in this environment
(~45 MB/s), not device execution (~1 ms).  The tunnel entropy-codes its
payload (measured: uniform bytes 46.6 MB/s, gaussian 52.5, half-entropy
62.9, zeros 70.3; time ~ fixed + 0.71*H), recovering only ~70% of any
entropy slack -- so explicit bit-packing beats shipping sparse codes and
letting the wire compress, and gaussian-distributed planes (h's int8
part, the output hi bytes) ride the discount automatically.
"""

import math
import os
import sys
from contextlib import ExitStack

import numpy as np

for _p in ("/opt/trn_rl_repo", "/root/.axon_site/_ro/trn_rl_repo"):
    if os.path.isdir(_p) and _p not in sys.path:
        sys.path.insert(0, _p)

import ml_dtypes

import concourse.bacc as bacc
import concourse.bass as bass
import concourse.mybir as mybir
import concourse.tile as tile
from concourse.bass_utils import run_bass_kernel_spmd

BF16 = ml_dtypes.bfloat16
F32 = np.float32

P = 128
NA = 50000          # atoms
NE = 800000         # edges
EMB = 128
NCORE = 8
APC = NA // NCORE   # atoms per core = 6250
WIN = 128           # scatter window (atoms) = onehot width
NWIN = (APC + WIN - 1) // WIN           # 49 windows/core
TILE = 512
TPC = 13            # phase-1 tiles per core (104 total >= 98 real)
NAPC = TPC * TILE   # 6656 atom slots computed per core
NAPG = NCORE * NAPC  # 53248 global padded table rows
TBL_SPLIT = 32768   # table row split so int16 gather indices stay in range
GCH = 64            # gather/bfT chunk size in 128-edge blocks

BITS = 5            # bf quantization bits (4, 5, or 6)
GROUP = {4: 2, 5: 8, 6: 4}[BITS]     # values per packed group
NBY = BITS * GROUP // 8              # bytes per packed group
QBF = float(1 << BITS)               # midrise: q=floor(bf*QBF), deq (q+.5)/QBF

OBITS = 10         # output bits per value (8, 10, or 12; per-feature scaled)
Q10 = 160.0         # h 10-bit quantization scale on h^T/S (clip ~5.3 sigma)
HRE = 6252          # h columns shipped per core (6250 real + 2 pad, mult 4)
H2B = HRE // 4      # bytes of packed 2-bit h low plane per partition (1563)
H2BP = H2B + 1      # padded to even (1564) so the int16 region stays aligned
SILU_S = 1.0 / 0.6
INV_SQRT2 = float(1.0 / math.sqrt(2.0))

dt = mybir.dt


def _ceil128(x):
    return (np.asarray(x, np.int64) + 127) // 128 * 128


def _atom_perm(a):
    """Atom id -> physical row in the h_res DRAM table.

    Each core ships HRE=6252 h columns (its 6250 atoms + 2 junk) and runs 13
    tiles of 512 over them; the last tile re-reads columns [5740, 6252), so
    tiles 11 and 12 overlap and either copy of a duplicated atom is valid --
    we index the natural r//512 one for r < 6144 and tile 12 for the tail.
    Phase 1 stores each 512-atom tile via 4 PE transposes packed contiguously
    per partition; row q = tile*512 + (rr%128)*4 + rr//128."""
    a = np.asarray(a, np.int64)
    c, r = a // APC, a % APC
    jlast = r >= 12 * 512
    j = np.where(jlast, 12, r // 512)
    rr = np.where(jlast, r - (HRE - 512), r - j * 512)
    return c * NAPC + j * 512 + (rr % 128) * 4 + rr // 128


def _balanced_bins(idx_t, deg0):
    """Partition atoms into NCORE*NWIN bins of <=WIN atoms with near-equal
    total in-degree (LPT greedy), then rank-match bins to (core, window) by
    low-half degree so per-core seg totals align across cores.

    Returns (acore, awin, atrel) per atom."""
    import heapq

    deg = np.bincount(np.asarray(idx_t, np.int64), minlength=NA)
    nbins = NCORE * NWIN
    order = np.argsort(deg, kind="stable")[::-1]
    heap = [(0, 0, b) for b in range(nbins)]
    heapq.heapify(heap)
    bin_of = np.empty(NA, np.int32)
    trel_of = np.empty(NA, np.int32)
    stash = []
    for a in order:
        while True:
            load, cnt_b, b = heapq.heappop(heap)
            if cnt_b < WIN:
                break
            stash.append((load, cnt_b, b))
        bin_of[a] = b
        trel_of[a] = cnt_b
        heapq.heappush(heap, (load + int(deg[a]), cnt_b + 1, b))
    # per-bin low-half degree
    b0 = np.bincount(bin_of, weights=deg0[np.arange(NA)].astype(np.float64),
                     minlength=nbins).astype(np.int64)
    rank = np.argsort(b0, kind="stable")[::-1]       # bins by cnt0 desc
    bin_core = np.empty(nbins, np.int32)
    bin_win = np.empty(nbins, np.int32)
    for wpos in range(NWIN):
        grp = rank[wpos * NCORE:(wpos + 1) * NCORE]
        bin_core[grp] = np.arange(NCORE)
        bin_win[grp] = wpos
    return (bin_core[bin_of], bin_win[bin_of], trel_of)


def pack_edges(idx_s, idx_t):
    """Host-side edge sharding/packing (contiguous per (core, seg), windows
    back-to-back with no per-window padding).  Returns the static schedule
    (identical across cores: block/window pair list with start/stop flags)
    plus per-core slot assignment of every real edge and the per-core
    (atom -> output column) maps."""
    idx_s = np.asarray(idx_s, np.int64)
    idx_t = np.asarray(idx_t, np.int64)
    pi = _atom_perm(idx_s)
    g = (pi >= TBL_SPLIT).astype(np.int64)
    deg0 = np.bincount(idx_t[g == 0], minlength=NA)

    acore, awin, atrel = _balanced_bins(idx_t, deg0)
    core = acore[idx_t].astype(np.int64)
    w = awin[idx_t].astype(np.int64)
    trel = atrel[idx_t].astype(np.int64)

    key = (core * 2 + g) * NWIN + w
    order = np.argsort(key, kind="stable")
    cnt = np.bincount(key, minlength=NCORE * 2 * NWIN).reshape(NCORE, 2, NWIN)

    # per-core contiguous offsets: seg0 windows at [0, LTOT), seg1 at
    # [CAPL, CAPL+HTOT); caps are the max over cores, rounded to 128
    LTOT = cnt[:, 0, :].sum(axis=1)
    HTOT = cnt[:, 1, :].sum(axis=1)
    CAPL = int(max(_ceil128(LTOT.max()), 128))
    CAPH = int(max(_ceil128(HTOT.max()), 128))
    EPAD = CAPL + CAPH
    NBLK = EPAD // 128

    woff = np.zeros((NCORE, 2, NWIN + 1), np.int64)
    woff[:, 0, 1:] = np.cumsum(cnt[:, 0, :], axis=1)
    woff[:, 1, 1:] = CAPL + np.cumsum(cnt[:, 1, :], axis=1)
    woff[:, 1, 0] = CAPL

    off_by_key = np.empty(NCORE * 2 * NWIN, np.int64)
    for c in range(NCORE):
        off_by_key[(c * 2 + 0) * NWIN:(c * 2 + 1) * NWIN] = woff[c, 0, :-1]
        off_by_key[(c * 2 + 1) * NWIN:(c * 2 + 2) * NWIN] = woff[c, 1, :-1]
    grp_start = np.concatenate([[0], np.cumsum(cnt.reshape(-1))])
    k_sorted = key[order]
    pos = np.arange(NE, dtype=np.int64) - grp_start[k_sorted]
    # slot in ORIGINAL edge order (avoids materializing permuted copies of
    # the big edge-feature array later)
    slot = np.empty(NE, np.int64)
    slot[order] = off_by_key[k_sorted] + pos

    # static (block, window) pair schedule: for each seg window, the union
    # over cores of blocks it intersects; on cores where a pair's block has
    # no edges of that window the (shipped, per-core) tcol mask is all -1
    # and the matmul adds zero
    pairs = []            # (seg, blk, w, start, stop)
    for seg, blkbase, blkend in ((0, 0, CAPL // 128), (1, CAPL // 128, NBLK)):
        for wi in range(NWIN):
            s = woff[:, seg, wi]
            e = woff[:, seg, wi + 1]
            nz = e > s
            if not nz.any():
                pairs.append((seg, blkbase, wi, True, True))  # empty chain
                continue
            sblk = int((s[nz] // 128).min())
            eblk = int(((e[nz] + 127) // 128).max())
            eblk = min(eblk, blkend) if seg == 1 else min(eblk, CAPL // 128)
            for b in range(sblk, eblk):
                pairs.append((seg, b, wi, b == sblk, b == eblk - 1))
    # device iterates pairs grouped by block: order by (block, window)
    pairs.sort(key=lambda t: (t[1], t[2]))
    NPAIR = len(pairs)

    # self-check: every edge's (block, window) is covered by a pair
    pair_exists = np.zeros((NBLK, NWIN), bool)
    for seg, b, wi, _, _ in pairs:
        pair_exists[b, wi] = True
    assert pair_exists[slot // 128, w].all(), "edge missing from pair schedule"
    # self-check: <=4 window chains open at once (psX pool depth)
    live = peak = 0
    for seg, b, wi, st, sp in pairs:
        if st:
            live += 1
            peak = max(peak, live)
        if sp:
            live -= 1
    assert live == 0 and peak <= 4, f"chain concurrency {peak} (live {live})"

    # per-core tcol plane: tcol[p, pair] = trel of slot blk*128+p if that
    # slot holds an edge of the pair's window on this core, else -1
    tcolp = np.full((NCORE, P, NPAIR), -1, np.int8)
    slot_w = np.full((NCORE, EPAD), -1, np.int16)
    slot_t = np.zeros((NCORE, EPAD), np.int8)
    slot_w[core, slot] = w.astype(np.int16)
    slot_t[core, slot] = trel.astype(np.int8)
    for pj, (seg, b, wi, _, _) in enumerate(pairs):
        sl = np.s_[b * 128:(b + 1) * 128]
        m = slot_w[:, sl] == wi        # [NCORE, 128]
        tcolp[:, :, pj] = np.where(m, slot_t[:, sl], -1)

    return dict(
        core=core, slot=slot, pi=pi, g=g,
        acore=acore, awin=awin, atrel=atrel,
        EPAD=EPAD, CAPL=CAPL, NBLK=NBLK,
        pairs=pairs, NPAIR=NPAIR, tcolp=tcolp,
    )


def build_host_inputs(h, bf, w_bf, w_pre, w_mlp1, w_res, scale_sum, pk):
    """Build the per-core in_maps (numpy arrays keyed by DRAM tensor name)."""
    S = SILU_S
    EPAD, NBLK = pk["EPAD"], pk["NBLK"]

    # folded weights, natural [in, out] layout; 10 slots of [128,128]:
    #  0: W0' = S*w_pre0       1: W1' = S*w_pre1
    #  2: Wm' = S*C*scale*w_mlp1        3: w_bf/QBF (bf dequant folded)
    #  4..9: Ai' = S*w_res[i,0], Bi' = S*w_res[i,1]
    scale = float(np.asarray(scale_sum).reshape(-1)[0])
    wl = [
        np.asarray(w_pre[0], F32) * S,
        np.asarray(w_pre[1], F32) * S,
        np.asarray(w_mlp1, F32) * (S * INV_SQRT2 * scale),
        np.asarray(w_bf, F32) * (1.0 / (2.0 * QBF)),
    ]
    for i in range(3):
        wl.append(np.asarray(w_res[i, 0], F32) * S)
        wl.append(np.asarray(w_res[i, 1], F32) * S)
    wts = np.concatenate([x.astype(BF16) for x in wl], axis=1)  # [128, 10*128]

    # h^T/S at 10 bits, scale Q10: v10 = 4*vhi + vlo; ship vhi int8 and a
    # packed 2-bit vlo plane (4 columns per byte)
    hT = np.zeros((P, NCORE * HRE), F32)
    hcols = np.asarray(h, F32).T  # [128, 50000]
    for c in range(NCORE):
        hT[:, c * HRE:c * HRE + APC] = hcols[:, c * APC:(c + 1) * APC]
    v10 = np.clip(np.rint(hT * (Q10 / S)), -512, 511).astype(np.int16)
    vhi = (v10 >> 2).astype(np.int8)            # floor division
    vlo = (v10 & 3).astype(np.uint8)
    vlo4 = vlo.reshape(P, NCORE * H2B, 4)
    hplane = (vlo4[:, :, 0] | (vlo4[:, :, 1] << 2) | (vlo4[:, :, 2] << 4)
              | (vlo4[:, :, 3] << 6)).astype(np.uint8)  # [P, NCORE*H2B]

    iota = np.ascontiguousarray(
        np.broadcast_to(np.arange(WIN, dtype=F32).astype(BF16), (P, WIN)))
    ident = np.eye(P, dtype=BF16)

    # bf -> BITS-bit midrise codes in chunks (values in [0,1))
    bf = np.asarray(bf, F32)
    bf_q = np.empty((NE, P), np.uint8)
    tmp = np.empty((100000, P), F32)
    for s in range(0, NE, 100000):
        e = min(s + 100000, NE)
        t = tmp[:e - s]
        np.multiply(bf[s:e], QBF, out=t)
        np.floor(t, out=t)
        bf_q[s:e] = np.minimum(t, QBF - 1).astype(np.uint8)

    ecore, slot = pk["core"], pk["slot"]
    bfr = np.zeros((NCORE, EPAD, P), np.uint8)
    bfr[ecore, slot] = bf_q

    gidx = np.zeros((NCORE, EPAD), np.int16)
    gidx[ecore, slot] = (pk["pi"] - pk["g"] * TBL_SPLIT).astype(np.int16)
    gidx = np.ascontiguousarray(
        gidx.reshape(NCORE, EPAD // 16, 16).transpose(0, 2, 1))  # [NCORE,16,EPAD//16]

    # per-(block, window) pair tcol planes from pack_edges; pairs the core
    # doesn't populate stay -1 (iota never matches -> zero contribution)
    tcolp = pk["tcolp"]          # [NCORE, P, NPAIR] int8
    NPAIR = pk["NPAIR"]

    # ONE blob tensor per core (a single device_put; per-put overhead on the
    # axon tunnel is ~80 ms).  Byte layout per partition row:
    #   [0, E3)              edge features BITS-bit packed (slot order,
    #                        features on partitions)
    #   [E3, +HRE)           h^T/S vhi int8 (this core's 6252 columns)
    #   [+H2BP)              h 2-bit low plane (padded to even)
    #   [+2*EPAD/128)        gather indices int16, 16-wrap flat as [128, E/128]
    #   [+NPAIR]             per-(block, window) pair target column int8
    #   [.., +2*AUXW/8)      bf16 aux slice (wts | iota | ident rows
    #                        [16c,16c+16) flat; AllGather rebuilds the block)
    E3 = EPAD * BITS // 8
    OFF_H = E3
    OFF_H2 = E3 + HRE
    OFF_G = OFF_H2 + H2BP
    OFF_T = OFF_G + EPAD // 64
    OFF_A = (OFF_T + NPAIR + 1) // 2 * 2   # bf16-aligned aux start
    AUXW = 10 * P + WIN + P
    AUXS = AUXW // NCORE                   # aux columns shipped per core
    W2 = OFF_A // 2 + AUXS
    ngrp = EPAD // GROUP
    shifts = [(k * BITS) for k in range(GROUP)]
    in_maps = []
    packed = np.empty((P, E3), np.uint8)
    for c in range(NCORE):
        blob = np.zeros((P, W2), BF16)
        b8u = blob.view(np.uint8)
        b8s = blob.view(np.int8)
        # pack GROUP consecutive slots' codes into NBY bytes (little-endian
        # bit order) along the free dim
        qT = np.ascontiguousarray(bfr[c].T)          # [P, EPAD] uint8
        qg = qT.reshape(P, ngrp, GROUP).astype(np.uint64)
        word = np.zeros((P, ngrp), np.uint64)
        for k, sh in enumerate(shifts):
            word |= qg[:, :, k] << sh
        pb = packed.reshape(P, ngrp, NBY)
        for j in range(NBY):
            pb[:, :, j] = (word >> (8 * j)).astype(np.uint8)
        b8u[:, :E3] = packed
        b8s[:, OFF_H:OFF_H2] = vhi[:, c * HRE:(c + 1) * HRE]
        b8u[:, OFF_H2:OFF_H2 + H2B] = hplane[:, c * H2B:(c + 1) * H2B]
        b16 = blob.view(np.int16)
        b16[:, OFF_G // 2:OFF_G // 2 + EPAD // 128] = \
            gidx[c].reshape(P, EPAD // 128)
        b8s[:, OFF_T:OFF_T + NPAIR] = tcolp[c]
        # aux (wts | iota | ident) is identical across cores: ship only this
        # core's 16-partition-row slice; an AllGather rebuilds the full
        # [128, AUXW] block on device
        CAUX = OFF_A // 2
        aux_full = np.concatenate([wts, iota, ident], axis=1)  # [128, AUXW]
        blob[:, CAUX:CAUX + AUXS] = \
            aux_full[16 * c:16 * (c + 1), :].reshape(P, AUXS)
        in_maps.append({"blob": blob})
    return in_maps


def chunks_static(pk):
    """Gather/bfT chunk list: (seg, b0, b1) block ranges within one table
    half, at most GCH blocks each."""
    segblk = pk["CAPL"] // 128
    chunks = []
    b = 0
    while b < pk["NBLK"]:
        seg = 0 if b < segblk else 1
        lim = segblk if seg == 0 else pk["NBLK"]
        e = min(b + GCH, lim)
        chunks.append((seg, b, e))
        b = e
    return chunks


def _unpack_ops(nc, src_ap, dst_ap, n4, tmp1, tmp2):
    """Emit DVE ops turning BITS-bit packed bytes into int8 codes.

    src_ap/dst_ap: AP factories f(byte_or_slot_offset) -> strided AP of n4
    elements per partition."""
    A = mybir.AluOpType
    mask = (1 << BITS) - 1
    for k in range(GROUP):
        bit0 = k * BITS
        j0, sh = bit0 // 8, bit0 % 8
        if sh + BITS <= 8:
            if sh == 0:
                nc.vector.tensor_scalar(dst_ap(k), src_ap(j0), mask, None,
                                        A.bitwise_and)
            else:
                nc.vector.tensor_scalar(dst_ap(k), src_ap(j0), sh, mask,
                                        A.logical_shift_right, A.bitwise_and)
        else:
            hi_bits = sh + BITS - 8
            nc.vector.tensor_scalar(tmp1[:, :n4], src_ap(j0), sh,
                                    (1 << (8 - sh)) - 1,
                                    A.logical_shift_right, A.bitwise_and)
            nc.vector.tensor_scalar(tmp2[:, :n4], src_ap(j0 + 1),
                                    (1 << hi_bits) - 1, 8 - sh,
                                    A.bitwise_and, A.logical_shift_left)
            nc.vector.tensor_tensor(dst_ap(k), tmp1[:, :n4], tmp2[:, :n4],
                                    A.add)


def build_bass(pk, enable_asserts=False, act_fn=None):
    EPAD, NBLK = pk["EPAD"], pk["NBLK"]
    chunks = chunks_static(pk)
    pairs = pk["pairs"]
    NPAIR = pk["NPAIR"]
    # pair index ranges per block (pairs are sorted by (block, window))
    pairs_of_block = [[] for _ in range(NBLK)]
    for pj, (seg, b, wi, st, sp) in enumerate(pairs):
        pairs_of_block[b].append(pj)
    ACT = act_fn or mybir.ActivationFunctionType.Silu

    nc = bacc.Bacc("TRN2", target_bir_lowering=False, debug=False,
                   enable_asserts=enable_asserts, num_devices=NCORE)

    E3 = EPAD * BITS // 8
    OFF_H = E3
    OFF_H2 = E3 + HRE
    OFF_G = OFF_H2 + H2BP
    OFF_T = OFF_G + EPAD // 64
    OFF_A = (OFF_T + NPAIR + 1) // 2 * 2
    AUXW = 10 * P + WIN + P
    AUXS = AUXW // NCORE
    W2 = OFF_A // 2 + AUXS
    blob = nc.dram_tensor("blob", [P, W2], dt.bfloat16,
                          kind="ExternalInput").ap()
    blob8 = blob[:, :].bitcast(dt.int8)      # [128, 2*W2] int8 view
    blob16 = blob[:, :].bitcast(dt.int16)    # [128, W2] int16 view
    aux = blob[:, OFF_A // 2:OFF_A // 2 + AUXS]
    # gather indices: virtual [16, EPAD/16] over the flat int16 region
    gidx = bass.AP(blob16.tensor, OFF_G // 2,
                   [[8 * W2, 16], [W2, 8], [1, EPAD // 128]])
    # output: per-feature OBITS-bit codes (hi byte plane + packed lo plane
    # for OBITS>8 + per-feature absmax f32)
    OC = NWIN * WIN
    OW = OC + {8: 0, 10: OC // 4, 12: OC // 2}[OBITS] + 4
    outt = nc.dram_tensor("outt", [P, OW], dt.int8,
                          kind="ExternalOutput").ap()

    with tile.TileContext(nc) as tc, ExitStack() as ctx:
        const = ctx.enter_context(tc.tile_pool(name="const", bufs=1))
        dram = ctx.enter_context(tc.tile_pool(name="dram", bufs=1, space="DRAM"))
        ph1 = ctx.enter_context(tc.tile_pool(name="ph1", bufs=3))
        edge = ctx.enter_context(tc.tile_pool(name="edge", bufs=2))
        xoh = ctx.enter_context(tc.tile_pool(name="xoh", bufs=4))
        mlp = ctx.enter_context(tc.tile_pool(name="mlp", bufs=2))
        psA = ctx.enter_context(tc.tile_pool(name="psA", bufs=2, space="PSUM"))
        # psW serves BOTH the phase-1 transposes (tag tp) and the phase-2
        # mlp_bf matmuls (tag mm) -- the phases are disjoint in time, so one
        # double-buffered 2-bank pool covers both (PSUM is only 8 banks)
        psW = ctx.enter_context(tc.tile_pool(name="psW", bufs=2, space="PSUM"))
        psX = ctx.enter_context(tc.tile_pool(name="psX", bufs=4, space="PSUM"))

        # aux (wts | iota | ident) arrives as this core's 1/8 slice; rebuild
        # the replicated [128, AUXW] block with a small AllGather: the slice
        # is the flat row-major image of aux rows [16c, 16c+16)
        agw = dram.tile([16, AUXW], dt.bfloat16, tag="agw")
        awall = dram.tile([P, AUXW], dt.bfloat16, tag="awall")
        aux_part = const.tile([P, AUXS], dt.bfloat16)
        nc.sync.dma_start(aux_part[:], aux)
        agw_ap = agw[:, :]
        nc.sync.dma_start(
            bass.AP(agw_ap.tensor, 0, [[AUXS, P], [1, AUXS]]), aux_part[:])
        tc.strict_bb_all_engine_barrier()
        nc.gpsimd.collective_compute(
            "AllGather", mybir.AluOpType.bypass,
            replica_groups=[list(range(NCORE))],
            ins=[agw[:, :].opt()], outs=[awall[:, :].opt()])
        tc.strict_bb_all_engine_barrier()
        aux_sb = const.tile([P, AUXW], dt.bfloat16)
        nc.sync.dma_start(aux_sb[:], awall[:, :])
        W = [aux_sb[:, i * P:(i + 1) * P] for i in range(10)]
        W0p, W1p, Wmp, Wbf = W[0], W[1], W[2], W[3]
        iota_sb = aux_sb[:, 10 * P:10 * P + WIN]
        ident_sb = aux_sb[:, 10 * P + WIN:10 * P + WIN + P]
        tcol8 = const.tile([P, NPAIR], dt.int8)
        nc.sync.dma_start(tcol8[:], blob8[:, OFF_T:OFF_T + NPAIR])
        tcol_sb = const.tile([P, NPAIR], dt.float32)
        nc.vector.tensor_copy(tcol_sb[:], tcol8[:])
        # gather indices arrive 16-wrapped; replicate to the 128-partition
        # layout the SWDGE gather engine expects
        gidx_sb = const.tile([P, EPAD // 16], dt.int16)
        for k in range(8):
            nc.sync.dma_start(gidx_sb[16 * k:16 * (k + 1), :], gidx)
        staging = const.tile([P, NWIN * WIN], dt.bfloat16)

        agin = dram.tile([NAPC, P], dt.bfloat16, tag="agin")
        table = dram.tile([NAPG, P], dt.bfloat16, tag="table")

        A = mybir.AluOpType

        # -------- phase 1: h_res table (sharded + AllGather) ---------------
        for i in range(TPC):
            lo = min(i * 512, HRE - 512)   # last tile re-reads [5740, 6252)
            h8 = ph1.tile([P, 512], dt.int8, tag="h8", name=f"h8_{i}")
            nc.sync.dma_start(
                h8[:], blob8[:, OFF_H + lo:OFF_H + lo + 512])
            hp = ph1.tile([P, P], dt.int8, tag="hp", name=f"hp_{i}")
            nc.sync.dma_start(
                hp[:], blob8[:, OFF_H2 + lo // 4:OFF_H2 + lo // 4 + P])
            hl = ph1.tile([P, 512], dt.int8, tag="hl", name=f"hl_{i}")
            hpa = hp[:, :]
            hla = hl[:, :]
            for k in range(4):
                dsta = bass.AP(hla.tensor, hla.offset + k,
                               [[hla.ap[0][0], P], [4, P]])
                if k == 0:
                    nc.vector.tensor_scalar(dsta, hpa, 3, None, A.bitwise_and)
                else:
                    nc.vector.tensor_scalar(dsta, hpa, 2 * k, 3,
                                            A.logical_shift_right,
                                            A.bitwise_and)
            hT = ph1.tile([P, 512], dt.bfloat16, tag="hT", name=f"hT{i}")
            nc.vector.tensor_scalar(hT[:], h8[:], 4.0 / Q10, None,
                                    mybir.AluOpType.mult)
            hlf = ph1.tile([P, 512], dt.bfloat16, tag="hlf", name=f"hlf{i}")
            nc.vector.tensor_scalar(hlf[:], hl[:], 1.0 / Q10, None,
                                    mybir.AluOpType.mult)
            nc.vector.tensor_add(hT[:], hT[:], hlf[:])
            p1 = psA.tile([P, 512], dt.float32, tag="p1", name=f"p1_{i}")
            nc.tensor.matmul(p1[:], W0p, hT[:], start=True, stop=True)
            y1 = ph1.tile([P, 512], dt.bfloat16, tag="y1", name=f"y1_{i}")
            nc.scalar.activation(y1[:], p1[:], ACT)
            p2 = psA.tile([P, 512], dt.float32, tag="p1", name=f"p2_{i}")
            nc.tensor.matmul(p2[:], W1p, y1[:], start=True, stop=True)
            y2 = ph1.tile([P, 512], dt.bfloat16, tag="y2", name=f"y2_{i}")
            nc.scalar.activation(y2[:], p2[:], ACT)
            tres = ph1.tile([P, 512], dt.bfloat16, tag="tres", name=f"tr_{i}")
            nc.vector.tensor_add(tres[:], hT[:], y2[:])
            tp = psW.tile([P, 512], dt.bfloat16, tag="w", name=f"tp_{i}")
            for t in range(4):
                nc.tensor.transpose(tp[:, t * P:(t + 1) * P],
                                    tres[:, t * P:(t + 1) * P], ident_sb)
            st = ph1.tile([P, 512], dt.bfloat16, tag="st", name=f"st_{i}")
            nc.vector.tensor_copy(st[:], tp[:])
            ag_ap = agin[:, :]
            dst = bass.AP(ag_ap.tensor, i * 512 * P, [[512, P], [1, 512]])
            nc.sync.dma_start(dst, st[:])

        # hard barriers around the AllGather: phase-1 writes must land in
        # agin before it ships, and no gather may read `table` before the
        # collective completes (belt-and-braces vs a missed dep edge;
        # costs ~us of device time)
        tc.strict_bb_all_engine_barrier()
        nc.gpsimd.collective_compute(
            "AllGather", mybir.AluOpType.bypass,
            replica_groups=[list(range(NCORE))],
            ins=[agin[:, :].opt()], outs=[table[:, :].opt()])
        tc.strict_bb_all_engine_barrier()

        # ---------------- phase 2: edge stream -----------------------------
        x2map = {}

        def finish_window(seg, w):
            sl = staging[:, w * WIN:(w + 1) * WIN]
            if seg == 0:
                nc.vector.tensor_copy(sl, x2map[w][:])
            else:
                nc.vector.tensor_add(sl, sl, x2map[w][:])
            del x2map[w]

        NBB = GCH * P * BITS // 8   # packed bytes per full chunk
        for ci, (seg, b0, b1) in enumerate(chunks):
            nb = b1 - b0
            Gt = edge.tile([P, GCH * P], dt.bfloat16, tag="G", name=f"G{ci}")
            gt_ap = Gt[:, :]
            g_out = bass.AP(gt_ap.tensor, gt_ap.offset,
                            [[gt_ap.ap[0][0], P], [P, nb], [1, P]])
            src = table[0:TBL_SPLIT, :] if seg == 0 else table[TBL_SPLIT:NAPG, :]
            nc.gpsimd.dma_gather(
                g_out, src, gidx_sb[:, b0 * 8:b1 * 8],
                num_idxs=nb * P, num_idxs_reg=nb * P, elem_size=P,
                single_packet=False)
            # BITS-bit edge features: DMA packed bytes, unpack via shift/mask
            bpb = P * BITS // 8     # packed bytes per 128-edge block
            B6 = edge.tile([P, NBB], dt.int8, tag="B6", name=f"B6{ci}")
            nc.sync.dma_start(B6[:, :nb * bpb], blob8[:, b0 * bpb:b1 * bpb])
            B8 = edge.tile([P, GCH * P], dt.int8, tag="B8", name=f"B8{ci}")
            n4 = nb * P // GROUP
            t1 = t2 = None
            if any((k * BITS) % 8 + BITS > 8 for k in range(GROUP)):
                t1 = edge.tile([P, GCH * P // GROUP], dt.int8, tag="t1",
                               name=f"t1{ci}")
                t2 = edge.tile([P, GCH * P // GROUP], dt.int8, tag="t2",
                               name=f"t2{ci}")
            b6a = B6[:, :]
            b8a = B8[:, :]

            def _in(j, b6a=b6a, n4=n4):
                return bass.AP(b6a.tensor, b6a.offset + j,
                               [[b6a.ap[0][0], P], [NBY, n4]])

            def _out(k, b8a=b8a, n4=n4):
                return bass.AP(b8a.tensor, b8a.offset + k,
                               [[b8a.ap[0][0], P], [GROUP, n4]])

            _unpack_ops(nc, _in, _out, n4, t1, t2)
            # codes -> bf16 as 2q+1 with integer scalars (exact whether the
            # ALU runs int or float); the 1/2^(BITS+1) is folded into Wbf
            Bt = edge.tile([P, GCH * P], dt.bfloat16, tag="B", name=f"B{ci}")
            nc.vector.tensor_scalar(Bt[:, :nb * P], B8[:, :nb * P], 2, 1,
                                    A.mult, A.add)

            for q0 in range(0, nb, 4):
                qn = min(4, nb - q0)
                mm = psW.tile([P, 512], dt.float32, tag="w",
                              name=f"mm{ci}_{q0}")
                for j in range(qn):
                    nc.tensor.matmul(
                        mm[:, j * P:(j + 1) * P],
                        Bt[:, (q0 + j) * P:(q0 + j + 1) * P],
                        Wbf, start=True, stop=True)
                xg = xoh.tile([P, 512], dt.bfloat16, tag="x",
                              name=f"x{ci}_{q0}")
                nc.vector.tensor_mul(xg[:, :qn * P],
                                     Gt[:, q0 * P:(q0 + qn) * P],
                                     mm[:, :qn * P])
                # (block, window) pairs of this 4-block group: consecutive
                # pair indices; onehots for up to 4 pairs per DVE op via
                # stride-0 broadcast APs: oh[p, i*W+e] = (iota[e] == tcol[p,
                # pj0+i]); each pair then scatters xg[its block] into its
                # window's PSUM accumulator
                gp = [pj for j in range(qn)
                      for pj in pairs_of_block[b0 + q0 + j]]
                for i0 in range(0, len(gp), 4):
                    bn = min(4, len(gp) - i0)
                    pj0 = gp[i0]
                    oh4 = xoh.tile([P, 512], dt.bfloat16, tag="oh",
                                   name=f"oh{ci}_{q0}_{i0}")
                    in0 = bass.AP(iota_sb.tensor, iota_sb.offset,
                                  [[iota_sb.ap[0][0], P], [0, bn], [1, WIN]])
                    tsl = tcol_sb[:, pj0:pj0 + bn]
                    in1 = bass.AP(tsl.tensor, tsl.offset,
                                  [[tsl.ap[0][0], P], [1, bn], [0, WIN]])
                    nc.vector.tensor_tensor(oh4[:, :bn * WIN], in0, in1,
                                            mybir.AluOpType.is_equal)
                    for i in range(bn):
                        pseg, pb, wi, first, last = pairs[gp[i0 + i]]
                        j = pb - b0 - q0
                        if first:
                            x2map[wi] = psX.tile([P, WIN], dt.float32,
                                                 tag="x2",
                                                 name=f"x2_{pseg}_{wi}")
                        nc.tensor.matmul(x2map[wi][:],
                                         xg[:, j * P:(j + 1) * P],
                                         oh4[:, i * WIN:(i + 1) * WIN],
                                         start=first, stop=last)
                        if last:
                            finish_window(pseg, wi)

        assert not x2map, f"unclosed window chains: {sorted(x2map)}"

        # hard barrier: every finish_window write to `staging` must land
        # before the phase-3 matmuls read it (belt-and-braces vs a missed
        # dep edge; costs ~us of device time)
        tc.strict_bb_all_engine_barrier()

        # ---------------- phase 3: atom MLP (transposed) --------------------
        wptr, gi = 0, 0
        while wptr < NWIN:
            nw = min(4, NWIN - wptr)
            ncols = nw * WIN
            col0 = wptr * WIN
            rhs = staging[:, col0:col0 + ncols]
            p3 = psA.tile([P, 512], dt.float32, tag="p1", name=f"p3_{gi}")
            nc.tensor.matmul(p3[:, :ncols], Wmp, rhs, start=True, stop=True)
            xv = mlp.tile([P, 512], dt.bfloat16, tag="mx", name=f"mx_{gi}")
            nc.scalar.activation(xv[:, :ncols], p3[:, :ncols],
                                 ACT)
            for i in range(3):
                Ai, Bi = W[4 + 2 * i], W[5 + 2 * i]
                pa = psA.tile([P, 512], dt.float32, tag="p1",
                              name=f"pa{gi}_{i}")
                nc.tensor.matmul(pa[:, :ncols], Ai, xv[:, :ncols],
                                 start=True, stop=True)
                ad = mlp.tile([P, 512], dt.bfloat16, tag="ad",
                              name=f"ad{gi}_{i}")
                nc.scalar.activation(ad[:, :ncols], pa[:, :ncols],
                                     ACT)
                pb = psA.tile([P, 512], dt.float32, tag="p1",
                              name=f"pb{gi}_{i}")
                nc.tensor.matmul(pb[:, :ncols], Bi, ad[:, :ncols],
                                 start=True, stop=True)
                bd = mlp.tile([P, 512], dt.bfloat16, tag="bd",
                              name=f"bd{gi}_{i}")
                nc.scalar.activation(bd[:, :ncols], pb[:, :ncols],
                                     ACT)
                tsum = mlp.tile([P, 512], dt.bfloat16, tag="ts",
                                name=f"ts{gi}_{i}")
                nc.vector.tensor_add(tsum[:, :ncols], xv[:, :ncols],
                                     bd[:, :ncols])
                if i < 2:
                    xv = mlp.tile([P, 512], dt.bfloat16, tag="mx",
                                  name=f"mx{gi}_{i}")
                    nc.vector.tensor_scalar(xv[:, :ncols], tsum[:, :ncols],
                                            INV_SQRT2, None,
                                            mybir.AluOpType.mult)
                else:
                    # final residual result overwrites the (already
                    # consumed) staging columns; the 12-bit encode pass
                    # below reads it from there
                    nc.vector.tensor_scalar(staging[:, col0:col0 + ncols],
                                            tsum[:, :ncols],
                                            INV_SQRT2 * SILU_S, None,
                                            mybir.AluOpType.mult)
            wptr += nw
            gi += 1

        # ------------- 12-bit output encode (per-feature scale) -------------
        # u = rint(out*2047/absmax + 2048) in [1, 4095]; hi = floor(u/16)
        # (exact via rint(u/16 - 0.46875)); lo = u - 16*hi in [0, 15];
        # ship hi-128 (int8), nibble-packed lo pairs - 128 (int8), absmax f32
        enc = ctx.enter_context(tc.tile_pool(name="enc", bufs=2))
        QMAX = {8: 127.0, 10: 511.0, 12: 2047.0}[OBITS]
        mx = const.tile([P, 4], dt.float32)
        nc.vector.tensor_reduce(mx[:, 0:1], staging[:, :],
                                axis=mybir.AxisListType.X,
                                op=A.max, apply_absolute_value=True)
        nc.vector.tensor_scalar(mx[:, 0:1], mx[:, 0:1], 1e-20, None, A.max)
        nc.vector.reciprocal(mx[:, 1:2], mx[:, 0:1])
        nc.vector.tensor_scalar(mx[:, 2:3], mx[:, 1:2], QMAX, None, A.mult)
        nc.sync.dma_start(outt[:, OW - 4:OW].bitcast(dt.float32),
                          mx[:, 0:1])
        ECH = 512
        for e0 in range(0, OC, ECH):
            en = min(ECH, OC - e0)
            if OBITS == 8:
                qf = enc.tile([P, ECH], dt.float32, tag="qf", name=f"qf{e0}")
                nc.vector.tensor_scalar(qf[:, :en], staging[:, e0:e0 + en],
                                        mx[:, 2:3], None, A.mult)
                q8 = enc.tile([P, ECH], dt.int8, tag="hi8", name=f"q8{e0}")
                nc.vector.tensor_copy(q8[:, :en], qf[:, :en])
                nc.sync.dma_start(outt[:, e0:e0 + en], q8[:, :en])
                continue
            if OBITS == 10:
                # u = rint(out*511/absmax + 512) in [1,1023]; hi=floor(u/4)
                # exactly via rint(u*0.25 - 0.375); lo = u - 4*hi in [0,3],
                # four lo values per byte via pure mult/add
                eq = en // 4
                qf = enc.tile([P, ECH], dt.float32, tag="qf", name=f"qf{e0}")
                nc.vector.tensor_scalar(qf[:, :en], staging[:, e0:e0 + en],
                                        mx[:, 2:3], 512.0, A.mult, A.add)
                u16 = enc.tile([P, ECH], dt.int16, tag="u16", name=f"u{e0}")
                nc.vector.tensor_copy(u16[:, :en], qf[:, :en])
                hif = enc.tile([P, ECH], dt.float32, tag="hif", name=f"hf{e0}")
                nc.vector.tensor_scalar(hif[:, :en], u16[:, :en], 0.25,
                                        -0.375, A.mult, A.add)
                hi8 = enc.tile([P, ECH], dt.int8, tag="hi8", name=f"h8{e0}")
                nc.vector.tensor_scalar(hi8[:, :en], hif[:, :en], 128.0,
                                        None, A.subtract)
                hr = enc.tile([P, ECH], dt.float32, tag="hr", name=f"hr{e0}")
                nc.vector.tensor_scalar(hr[:, :en], hi8[:, :en], 4.0, 512.0,
                                        A.mult, A.add)
                lo = enc.tile([P, ECH], dt.float32, tag="lo", name=f"lo{e0}")
                nc.vector.tensor_tensor(lo[:, :en], u16[:, :en], hr[:, :en],
                                        A.subtract)
                loa = lo[:, :]

                def _los(k, loa=loa, eq=eq):
                    return bass.AP(loa.tensor, loa.offset + k,
                                   [[loa.ap[0][0], P], [4, eq]])

                s1 = enc.tile([P, ECH // 2], dt.float32, tag="s1",
                              name=f"s1{e0}")
                s2 = enc.tile([P, ECH // 2], dt.float32, tag="s2",
                              name=f"s2{e0}")
                nc.vector.tensor_scalar(s1[:, :eq], _los(3), 64.0, 128.0,
                                        A.mult, A.subtract)
                nc.vector.tensor_scalar(s2[:, :eq], _los(2), 16.0, None,
                                        A.mult)
                nc.vector.tensor_tensor(s1[:, :eq], s1[:, :eq], s2[:, :eq],
                                        A.add)
                nc.vector.tensor_scalar(s2[:, :eq], _los(1), 4.0, None,
                                        A.mult)
                nc.vector.tensor_tensor(s1[:, :eq], s1[:, :eq], s2[:, :eq],
                                        A.add)
                nc.vector.tensor_tensor(s1[:, :eq], s1[:, :eq], _los(0),
                                        A.add)
                lob = enc.tile([P, ECH // 4], dt.int8, tag="lob",
                               name=f"lb{e0}")
                nc.vector.tensor_copy(lob[:, :eq], s1[:, :eq])
                nc.sync.dma_start(outt[:, e0:e0 + en], hi8[:, :en])
                nc.sync.dma_start(outt[:, OC + e0 // 4:OC + e0 // 4 + eq],
                                  lob[:, :eq])
                continue
            eh = en // 2
            qf = enc.tile([P, ECH], dt.float32, tag="qf", name=f"qf{e0}")
            nc.vector.tensor_scalar(qf[:, :en], staging[:, e0:e0 + en],
                                    mx[:, 2:3], 2048.0, A.mult, A.add)
            u16 = enc.tile([P, ECH], dt.int16, tag="u16", name=f"u{e0}")
            nc.vector.tensor_copy(u16[:, :en], qf[:, :en])
            hif = enc.tile([P, ECH], dt.float32, tag="hif", name=f"hf{e0}")
            nc.vector.tensor_scalar(hif[:, :en], u16[:, :en], 0.0625,
                                    -0.46875, A.mult, A.add)
            hi8 = enc.tile([P, ECH], dt.int8, tag="hi8", name=f"h8{e0}")
            nc.vector.tensor_scalar(hi8[:, :en], hif[:, :en], 128.0, None,
                                    A.subtract)
            hr = enc.tile([P, ECH], dt.float32, tag="hr", name=f"hr{e0}")
            nc.vector.tensor_scalar(hr[:, :en], hi8[:, :en], 16.0, 2048.0,
                                    A.mult, A.add)
            lo = enc.tile([P, ECH], dt.float32, tag="lo", name=f"lo{e0}")
            nc.vector.tensor_tensor(lo[:, :en], u16[:, :en], hr[:, :en],
                                    A.subtract)
            loa = lo[:, :]
            lo_ev = bass.AP(loa.tensor, loa.offset, [[loa.ap[0][0], P],
                                                     [2, eh]])
            lo_od = bass.AP(loa.tensor, loa.offset + 1, [[loa.ap[0][0], P],
                                                         [2, eh]])
            s1 = enc.tile([P, ECH // 2], dt.float32, tag="s1", name=f"s1{e0}")
            nc.vector.tensor_scalar(s1[:, :eh], lo_od, 16.0, 128.0,
                                    A.mult, A.subtract)
            s2 = enc.tile([P, ECH // 2], dt.float32, tag="s2", name=f"s2{e0}")
            nc.vector.tensor_tensor(s2[:, :eh], s1[:, :eh], lo_ev, A.add)
            lob = enc.tile([P, ECH // 2], dt.int8, tag="lob", name=f"lb{e0}")
            nc.vector.tensor_copy(lob[:, :eh], s2[:, :eh])
            nc.sync.dma_start(outt[:, e0:e0 + en], hi8[:, :en])
            nc.sync.dma_start(outt[:, OC + e0 // 2:OC + e0 // 2 + eh],
                              lob[:, :eh])

    nc.compile()
    return nc


def prepare(h, bf, idx_s, idx_t, w_bf, w_pre, w_mlp1, w_res, scale_sum,
            enable_asserts=False):
    """Pack inputs + build the compiled SPMD program. Returns (nc, in_maps)."""
    pk = pack_edges(idx_s, idx_t)
    in_maps = build_host_inputs(np.asarray(h), np.asarray(bf),
                                np.asarray(w_bf), np.asarray(w_pre),
                                np.asarray(w_mlp1), np.asarray(w_res),
                                np.asarray(scale_sum), pk)
    nc = build_bass(pk, enable_asserts=enable_asserts)
    return nc, in_maps, pk


def unshard_output(per_core_outt, pk):
    """Decode the OBITS-bit output planes and invert the balanced
    atom -> (core, window, pos) assignment."""
    OC = NWIN * WIN
    out = np.empty((NA, EMB), np.float32)
    acore, awin, atrel = pk["acore"], pk["awin"], pk["atrel"]
    cols = awin.astype(np.int64) * WIN + atrel
    for c in range(NCORE):
        r = np.asarray(per_core_outt[c])
        absmax = r[:, -4:].copy().view(np.float32)
        if OBITS == 8:
            t = r[:, :OC].astype(np.float32) * (absmax / 127.0)
        elif OBITS == 10:
            hi = r[:, :OC].astype(np.int32) + 128
            lob = r[:, OC:OC + OC // 4].astype(np.int32) + 128
            lo = np.empty((P, OC), np.int32)
            for k in range(4):
                lo[:, k::4] = (lob >> (2 * k)) & 3
            t = (4 * hi + lo - 512).astype(np.float32) * (absmax / 511.0)
        else:
            hi = r[:, :OC].astype(np.int32) + 128
            lob = r[:, OC:OC + OC // 2].astype(np.int32) + 128
            lo = np.empty((P, OC), np.int32)
            lo[:, 0::2] = lob & 15
            lo[:, 1::2] = lob >> 4
            t = (16 * hi + lo - 2048).astype(np.float32) * (absmax / 2047.0)
        ids = np.nonzero(acore == c)[0]
        out[ids] = t[:, cols[ids]].T
    return out


def kernel(h, bf, idx_s, idx_t, w_bf, w_pre, w_mlp1, w_res, scale_sum):
    nc, in_maps, pk = prepare(h, bf, idx_s, idx_t, w_bf, w_pre, w_mlp1, w_res,
                              scale_sum)
    res = run_bass_kernel_spmd(nc, in_maps, list(range(NCORE)))
    return unshard_output([res.results[c]["outt"] for c in range(NCORE)], pk)
